# revision 16
# baseline (speedup 1.0000x reference)
"""Trainium2 Bass kernel for sigmoid-attention (nn_Attention_40037685134082).

Contract: kernel(x, w_qkv, w_proj) -> (attn_matrix, attn_times_v, attn_proj)
matching reference.reference(). Inputs are the FULL tensors; sharding over the
8 NeuronCores happens inside: core c handles batch c//4 and the 4 heads
[4*(c%4), 4*(c%4)+4).

Per-core design (memory-bound problem; the attn matrix write dominates):
  - all inputs host-pre-transposed so no on-device transposes are needed,
    and pre-cast to fp16 (2^-11 rounding, ~the same error level as the
    hardware's relaxed-fp32 matmul mode)
  - attn matrix is computed transposed per head (attnT[m, n] = attn[n, m]);
    that orientation feeds the attn@V matmul directly and the host transposes
    while assembling the full output
  - all device matmuls run in fp16 (1 cycle/row on the PE, fast weight loads)
    with fp32 PSUM accumulation; outputs ship as fp16 and are widened to
    fp32 on the host (halves the dominant DMA traffic)
"""

import numpy as np

import concourse.bass as bass  # noqa: F401
import concourse.mybir as mybir
from concourse import bacc, tile
from concourse.bass_utils import run_bass_kernel_spmd

B, N, C, H = 2, 2048, 1024, 16
DH = C // H                      # 64
HPC = 4                          # heads per core
NCORES = 8
ATTN_BIAS = -7.625
SCALE = DH ** -0.5               # 0.125 (exact power of two; folded into wq)

F32 = mybir.dt.float32
F16 = mybir.dt.float16

BLK = 512                        # matmul free-dim block (1 PSUM bank f32 out)
NB = N // BLK                    # 4
NB2 = N // 1024                  # 2 (sigmoid / DMA granularity)
MT = N // 128                    # 16 m-tiles
CCH = C // 128                   # 8 contraction chunks over C


def build_nc():
    nc = bacc.Bacc("TRN2", target_bir_lowering=False, debug=False)

    xT_d = nc.dram_tensor("xT", [C, N], F16, kind="ExternalInput")
    wqT_d = nc.dram_tensor("wqT", [C, HPC * DH], F16, kind="ExternalInput")
    wkT_d = nc.dram_tensor("wkT", [C, HPC * DH], F16, kind="ExternalInput")
    wvT_d = nc.dram_tensor("wvT", [C, HPC * DH], F16, kind="ExternalInput")
    wpT_d = nc.dram_tensor("wpT", [HPC * DH, C], F16, kind="ExternalInput")

    attnT_d = nc.dram_tensor("attnT", [HPC, N, N], F16, kind="ExternalOutput")
    atvT_d = nc.dram_tensor("atvT", [HPC * DH, N], F16, kind="ExternalOutput")
    projT_d = nc.dram_tensor("projT", [C, N], F16, kind="ExternalOutput")

    with tile.TileContext(nc) as tc:
        with (
            tc.tile_pool(name="big", bufs=8) as big,      # xT chunks (4 KiB/part)
            tc.tile_pool(name="wgt", bufs=3) as wgt,      # qkv weights
            tc.tile_pool(name="att", bufs=16) as att,     # attnT tiles + staging (2 KiB/part)
            tc.tile_pool(name="sb", bufs=1) as sb,        # persistents
            tc.tile_pool(name="ps", bufs=2, space="PSUM") as ps,      # 2x2-bank slots
            tc.tile_pool(name="psav", bufs=2, space="PSUM") as psav,  # 2x2-bank slots
        ):
            # ---------------- input loads ----------------
            wq_s = wgt.tile([128, CCH, 256], F16, name="wq_s", tag="wgt")
            wk_s = wgt.tile([128, CCH, 256], F16, name="wk_s", tag="wgt")
            wv_s = wgt.tile([128, CCH, 256], F16, name="wv_s", tag="wgt")
            nc.sync.dma_start(wq_s[:], wqT_d.rearrange("(c p) d -> p c d", p=128))
            nc.sync.dma_start(wk_s[:], wkT_d.rearrange("(c p) d -> p c d", p=128))

            xT_r = xT_d.rearrange("(c p) n -> c p n", p=128)
            xs = []
            for ci in range(CCH):
                xt = big.tile([128, N], F16, name=f"xt{ci}", tag="big")
                nc.sync.dma_start(xt[:], xT_r[ci])
                xs.append(xt)

            nc.sync.dma_start(wv_s[:], wvT_d.rearrange("(c p) d -> p c d", p=128))
            # w_proj.T rows per head at partition base 0
            wp_s = sb.tile([64, HPC, C], F16, name="wp_s")
            nc.sync.dma_start(wp_s[:], wpT_d.rearrange("(h p) d -> p h d", p=64))

            bias_s = sb.tile([128, 1], F32, name="bias_s")
            nc.gpsimd.memset(bias_s[:], ATTN_BIAS)

            # persistents
            qT_s = sb.tile([128, 2, N], F16, name="qT_s")
            kT_s = sb.tile([128, 2, N], F16, name="kT_s")
            v_s = sb.tile([128, MT, 256], F16, name="v_s")
            atv_s = sb.tile([64, HPC, N], F16, name="atv_s")  # feeds proj

            # ---------------- qkv projections ----------------
            def emit_qk_pair(j):
                for w_s, outT in ((wq_s, qT_s), (wk_s, kT_s)):
                    for nb in range(NB):
                        pt = ps.tile([128, BLK], F32, name="mmps", tag="ps")
                        for ci in range(CCH):
                            nc.tensor.matmul(
                                pt[:],
                                w_s[:, ci, j * 128:(j + 1) * 128],
                                xs[ci][:, nb * BLK:(nb + 1) * BLK],
                                start=(ci == 0),
                                stop=(ci == CCH - 1),
                            )
                        nc.vector.tensor_copy(
                            outT[:, j, nb * BLK:(nb + 1) * BLK], pt[:]
                        )


            # ---------------- attention ----------------
            # Heads of a pair interleave: their K=64 QK matmuls sit on
            # different PE row-groups (partition bases 0 / 64) and overlap,
            # and their A@V accumulations live in separate PSUM banks.
            def emit_head_pair(hp, n2, with_v=False):
                    heads = (2 * hp, 2 * hp + 1)
                    n0 = n2 * 1024
                    pavs = []
                    for h in heads:
                        pav = psav.tile([64, 1024], F32, name=f"pav{h}_{n2}", tag="psav")
                        pavs.append(pav)
                    for mi in range(MT):
                        if with_v:
                            pt = ps.tile([128, 256], F32, name="mmps", tag="ps")
                            for ci in range(CCH):
                                nc.tensor.matmul(
                                    pt[:],
                                    xs[ci][:, mi * 128:(mi + 1) * 128],
                                    wv_s[:, ci, :],
                                    start=(ci == 0),
                                    stop=(ci == CCH - 1),
                                )
                            nc.vector.tensor_copy(v_s[:, mi, :], pt[:])
                        ats, pts = [], []
                        for ii, h in enumerate(heads):
                            hb = ii * 64
                            at = att.tile([128, 1024], F16, name=f"at{h}_{mi}", tag="att")
                            pt = ps.tile([128, 1024], F32, name="qkps", tag="ps")
                            for half in range(2):
                                c0 = n0 + half * BLK
                                # scoresT[m_tile, nblk] = kT_h[:, m].T @ qT_h[:, n]
                                nc.tensor.matmul(
                                    pt[:, half * BLK:(half + 1) * BLK],
                                    kT_s[hb:hb + 64, hp, mi * 128:(mi + 1) * 128],
                                    qT_s[hb:hb + 64, hp, c0:c0 + BLK],
                                    start=True,
                                    stop=True,
                                )
                            ats.append(at)
                            pts.append(pt)
                        for ii, h in enumerate(heads):
                            # attnT = sigmoid(scoresT + bias), PSUM -> SBUF fp16
                            nc.scalar.activation(
                                ats[ii][:], pts[ii][:],
                                mybir.ActivationFunctionType.Sigmoid,
                                bias=bias_s[:, 0:1],
                            )
                            for half in range(2):
                                # atvT_h[d, n] += v[m, d].T @ attnT[m, n]
                                nc.tensor.matmul(
                                    pavs[ii][:, half * BLK:(half + 1) * BLK],
                                    v_s[:, mi, h * 64:(h + 1) * 64],
                                    ats[ii][:, half * BLK:(half + 1) * BLK],
                                    start=(mi == 0),
                                    stop=(mi == MT - 1),
                                )
                            nc.sync.dma_start(
                                attnT_d[h, mi * 128:(mi + 1) * 128, n0:n0 + 1024],
                                ats[ii][:],
                            )
                    for ii, h in enumerate(heads):
                        # attn_times_v: stage to fp16 and DMA out; also keep for proj
                        avst = att.tile([64, 1024], F16, name=f"avst{h}_{n2}", tag="att")
                        nc.vector.tensor_copy(avst[:], pavs[ii][:])
                        nc.sync.dma_start(
                            atvT_d[h * 64:(h + 1) * 64, n0:n0 + 1024], avst[:]
                        )
                        nc.vector.tensor_copy(atv_s[:, h, n0:n0 + 1024], pavs[ii][:])

            # ---------------- output projection (partial over our 256 c_in) ----------------
            projT_r = projT_d.rearrange("(t p) n -> t p n", p=128)

            def emit_proj(n2):
                for ti in range(CCH):
                    st = att.tile([128, 1024], F16, name=f"st{ti}_{n2}", tag="att")
                    for half in range(2):
                        nb = n2 * 2 + half
                        pp = ps.tile([128, BLK], F32, name="qkps", tag="ps")
                        for hh in range(HPC):
                            nc.tensor.matmul(
                                pp[:],
                                wp_s[:, hh, ti * 128:(ti + 1) * 128],
                                atv_s[:, hh, nb * BLK:(nb + 1) * BLK],
                                start=(hh == 0),
                                stop=(hh == HPC - 1),
                            )
                        nc.vector.tensor_copy(
                            st[:, half * BLK:(half + 1) * BLK], pp[:]
                        )
                    nc.sync.dma_start(
                        projT_r[ti][:, n2 * 1024:(n2 + 1) * 1024], st[:]
                    )

            emit_qk_pair(0)
            emit_head_pair(0, 0, with_v=True)
            emit_head_pair(0, 1)
            emit_qk_pair(1)
            emit_head_pair(1, 0)
            emit_proj(0)
            emit_head_pair(1, 1)
            emit_proj(1)


    nc.compile()
    return nc


_NC_CACHE = None


def _get_nc():
    global _NC_CACHE
    if _NC_CACHE is None:
        _NC_CACHE = build_nc()
    return _NC_CACHE


def make_in_maps(x, w_qkv, w_proj):
    in_maps = []
    for c in range(NCORES):
        b, g = divmod(c, 4)
        r0 = g * HPC * DH  # 256-row slice for our 4 heads
        in_maps.append({
            "xT": np.ascontiguousarray(x[b].T.astype(np.float16)),
            "wqT": np.ascontiguousarray((w_qkv[r0:r0 + 256] * SCALE).T.astype(np.float16)),
            "wkT": np.ascontiguousarray(w_qkv[C + r0:C + r0 + 256].T.astype(np.float16)),
            "wvT": np.ascontiguousarray(w_qkv[2 * C + r0:2 * C + r0 + 256].T.astype(np.float16)),
            "wpT": np.ascontiguousarray(w_proj[:, r0:r0 + 256].T.astype(np.float16)),
        })
    return in_maps


def assemble(results):
    attn = np.empty((B, H, N, N), np.float32)
    atv = np.empty((B, N, C), np.float32)
    proj = np.empty((B, N, C), np.float32)
    for c in range(NCORES):
        b, g = divmod(c, 4)
        for j in range(HPC):
            attn[b, HPC * g + j] = results[c]["attnT"][j].T
        atv[b][:, g * 256:(g + 1) * 256] = results[c]["atvT"].T
    for b in range(B):
        acc = results[4 * b]["projT"].astype(np.float32)
        for g in range(1, 4):
            acc += results[4 * b + g]["projT"]
        proj[b] = acc.T
    return attn, atv, proj


def kernel(x, w_qkv, w_proj):
    x = np.asarray(x, dtype=np.float32)
    w_qkv = np.asarray(w_qkv, dtype=np.float32)
    w_proj = np.asarray(w_proj, dtype=np.float32)
    nc = _get_nc()
    in_maps = make_in_maps(x, w_qkv, w_proj)
    res = run_bass_kernel_spmd(nc, in_maps, list(range(NCORES))).results
    return assemble(res)


# revision 17
# speedup vs baseline: 1.6402x; 1.6402x over previous
"""Trainium2 Bass kernel for sigmoid-attention (nn_Attention_40037685134082).

Contract: kernel(x, w_qkv, w_proj) -> (attn_matrix, attn_times_v, attn_proj)
matching reference.reference(). Inputs are the FULL tensors; sharding over the
8 NeuronCores happens inside: core c handles batch c//4 and the 4 heads
[4*(c%4), 4*(c%4)+4).

Per-core design (memory-bound problem; the attn matrix write dominates):
  - all inputs host-pre-transposed so no on-device transposes are needed,
    and pre-cast to fp16 (2^-11 rounding, ~the same error level as the
    hardware's relaxed-fp32 matmul mode)
  - attn matrix is computed transposed per head (attnT[m, n] = attn[n, m]);
    that orientation feeds the attn@V matmul directly and the host transposes
    while assembling the full output
  - all device matmuls run in fp16 (1 cycle/row on the PE, fast weight loads)
    with fp32 PSUM accumulation; outputs ship as fp16 and are widened to
    fp32 on the host (halves the dominant DMA traffic)
"""

import numpy as np

import concourse.bass as bass  # noqa: F401
import concourse.mybir as mybir
from concourse import bacc, tile
from concourse.bass_utils import run_bass_kernel_spmd

B, N, C, H = 2, 2048, 1024, 16
DH = C // H                      # 64
HPC = 4                          # heads per core
NCORES = 8
ATTN_BIAS = -7.625
SCALE = DH ** -0.5               # 0.125 (exact power of two; folded into wq)

F32 = mybir.dt.float32
F16 = mybir.dt.float16

BLK = 512                        # matmul free-dim block (1 PSUM bank f32 out)
NB = N // BLK                    # 4
NB2 = N // 1024                  # 2 (sigmoid / DMA granularity)
MT = N // 128                    # 16 m-tiles
CCH = C // 128                   # 8 contraction chunks over C


def build_nc():
    nc = bacc.Bacc("TRN2", target_bir_lowering=False, debug=False)

    xT_d = nc.dram_tensor("xT", [C, N], F16, kind="ExternalInput")
    wqT_d = nc.dram_tensor("wqT", [C, HPC * DH], F16, kind="ExternalInput")
    wkT_d = nc.dram_tensor("wkT", [C, HPC * DH], F16, kind="ExternalInput")
    wvT_d = nc.dram_tensor("wvT", [C, HPC * DH], F16, kind="ExternalInput")
    wpT_d = nc.dram_tensor("wpT", [HPC * DH, C], F16, kind="ExternalInput")

    attnT_d = nc.dram_tensor("attnT", [HPC, N, N], F16, kind="ExternalOutput")
    atvT_d = nc.dram_tensor("atvT", [HPC * DH, N], F16, kind="ExternalOutput")
    projT_d = nc.dram_tensor("projT", [C, N], F16, kind="ExternalOutput")

    with tile.TileContext(nc) as tc:
        with (
            tc.tile_pool(name="big", bufs=8) as big,      # xT chunks (4 KiB/part)
            tc.tile_pool(name="wgt", bufs=3) as wgt,      # qkv weights
            tc.tile_pool(name="att", bufs=10) as att,     # attnT tiles + staging (2 KiB/part)
            tc.tile_pool(name="sb", bufs=1) as sb,        # persistents
            tc.tile_pool(name="ps", bufs=2, space="PSUM") as ps,      # 2x2-bank slots
            tc.tile_pool(name="psav", bufs=2, space="PSUM") as psav,  # 2x2-bank slots
        ):
            # ---------------- input loads ----------------
            xT_r = xT_d.rearrange("(c p) n -> c p n", p=128)
            xs = []
            for ci in range(CCH):
                xt = big.tile([128, N], F16, name=f"xt{ci}", tag="big")
                nc.sync.dma_start(xt[:], xT_r[ci])
                xs.append(xt)

            wq_s = wgt.tile([128, CCH, 256], F16, name="wq_s", tag="wgt")
            wk_s = wgt.tile([128, CCH, 256], F16, name="wk_s", tag="wgt")
            wv_s = wgt.tile([128, CCH, 256], F16, name="wv_s", tag="wgt")
            nc.sync.dma_start(wq_s[:], wqT_d.rearrange("(c p) d -> p c d", p=128))
            nc.sync.dma_start(wk_s[:], wkT_d.rearrange("(c p) d -> p c d", p=128))
            nc.sync.dma_start(wv_s[:], wvT_d.rearrange("(c p) d -> p c d", p=128))
            # w_proj.T rows per head at partition base 0
            wp_s = sb.tile([64, HPC, C], F16, name="wp_s")
            nc.sync.dma_start(wp_s[:], wpT_d.rearrange("(h p) d -> p h d", p=64))

            bias_s = sb.tile([128, 1], F32, name="bias_s")
            nc.gpsimd.memset(bias_s[:], ATTN_BIAS)

            # persistents
            qT_s = sb.tile([128, 2, N], F16, name="qT_s")
            kT_s = sb.tile([128, 2, N], F16, name="kT_s")
            v_s = sb.tile([128, MT, 256], F16, name="v_s")
            atv_s = sb.tile([64, HPC, N], F16, name="atv_s")  # feeds proj

            # ---------------- qkv projections ----------------
            def emit_qk_pair(j):
                for w_s, outT in ((wq_s, qT_s), (wk_s, kT_s)):
                    for nb in range(NB):
                        pt = ps.tile([128, BLK], F32, name="mmps", tag="ps")
                        for ci in range(CCH):
                            nc.tensor.matmul(
                                pt[:],
                                w_s[:, ci, j * 128:(j + 1) * 128],
                                xs[ci][:, nb * BLK:(nb + 1) * BLK],
                                start=(ci == 0),
                                stop=(ci == CCH - 1),
                            )
                        nc.vector.tensor_copy(
                            outT[:, j, nb * BLK:(nb + 1) * BLK], pt[:]
                        )

            emit_qk_pair(0)

            for mi in range(MT):
                pt = ps.tile([128, 256], F32, name="mmps", tag="ps")
                for ci in range(CCH):
                    nc.tensor.matmul(
                        pt[:],
                        xs[ci][:, mi * 128:(mi + 1) * 128],
                        wv_s[:, ci, :],
                        start=(ci == 0),
                        stop=(ci == CCH - 1),
                    )
                nc.vector.tensor_copy(v_s[:, mi, :], pt[:])

            # ---------------- attention ----------------
            def emit_head(h):
                hp, idx = divmod(h, 2)
                hb = idx * 64
                for n2 in range(NB2):
                    n0 = n2 * 1024
                    pav = psav.tile([64, 1024], F32, name="pav", tag="psav")
                    for mi in range(MT):
                        at = att.tile([128, 1024], F16, name=f"at{h}_{mi}", tag="att")
                        pt = ps.tile([128, 1024], F32, name="qkps", tag="ps")
                        for half in range(2):
                            c0 = n0 + half * BLK
                            # scoresT[m_tile, nblk] = kT_h[:, m].T @ qT_h[:, n]
                            nc.tensor.matmul(
                                pt[:, half * BLK:(half + 1) * BLK],
                                kT_s[hb:hb + 64, hp, mi * 128:(mi + 1) * 128],
                                qT_s[hb:hb + 64, hp, c0:c0 + BLK],
                                start=True,
                                stop=True,
                            )
                        # attnT = sigmoid(scoresT + bias), PSUM -> SBUF fp16
                        nc.scalar.activation(
                            at[:], pt[:],
                            mybir.ActivationFunctionType.Sigmoid,
                            bias=bias_s[:, 0:1],
                        )
                        for half in range(2):
                            # atvT_h[d, n] += v[m_chunk, d].T @ attnT[m_chunk, n]
                            nc.tensor.matmul(
                                pav[:, half * BLK:(half + 1) * BLK],
                                v_s[:, mi, h * 64:(h + 1) * 64],
                                at[:, half * BLK:(half + 1) * BLK],
                                start=(mi == 0),
                                stop=(mi == MT - 1),
                            )
                        nc.sync.dma_start(
                            attnT_d[h, mi * 128:(mi + 1) * 128, n0:n0 + 1024],
                            at[:],
                        )
                    # attn_times_v: stage to fp16 and DMA out; also keep for proj
                    avst = att.tile([64, 1024], F16, name=f"avst{h}_{n2}", tag="att")
                    nc.vector.tensor_copy(avst[:], pav[:])
                    nc.sync.dma_start(
                        atvT_d[h * 64:(h + 1) * 64, n0:n0 + 1024], avst[:]
                    )
                    nc.vector.tensor_copy(atv_s[:, h, n0:n0 + 1024], pav[:])

            emit_head(0)
            emit_qk_pair(1)
            emit_head(1)
            emit_head(2)
            emit_head(3)

            # ---------------- output projection (partial over our 256 c_in) ----------------
            projT_r = projT_d.rearrange("(t p) n -> t p n", p=128)
            for ti in range(CCH):
                for n2 in range(NB2):
                    st = att.tile([128, 1024], F16, name=f"st{ti}_{n2}", tag="att")
                    for half in range(2):
                        nb = n2 * 2 + half
                        pp = ps.tile([128, BLK], F32, name="qkps", tag="ps")
                        for hh in range(HPC):
                            nc.tensor.matmul(
                                pp[:],
                                wp_s[:, hh, ti * 128:(ti + 1) * 128],
                                atv_s[:, hh, nb * BLK:(nb + 1) * BLK],
                                start=(hh == 0),
                                stop=(hh == HPC - 1),
                            )
                        nc.vector.tensor_copy(
                            st[:, half * BLK:(half + 1) * BLK], pp[:]
                        )
                    nc.sync.dma_start(
                        projT_r[ti][:, n2 * 1024:(n2 + 1) * 1024], st[:]
                    )

    nc.compile()
    return nc


_NC_CACHE = None


def _get_nc():
    global _NC_CACHE
    if _NC_CACHE is None:
        _NC_CACHE = build_nc()
    return _NC_CACHE


def make_in_maps(x, w_qkv, w_proj):
    in_maps = []
    for c in range(NCORES):
        b, g = divmod(c, 4)
        r0 = g * HPC * DH  # 256-row slice for our 4 heads
        in_maps.append({
            "xT": np.ascontiguousarray(x[b].T.astype(np.float16)),
            "wqT": np.ascontiguousarray((w_qkv[r0:r0 + 256] * SCALE).T.astype(np.float16)),
            "wkT": np.ascontiguousarray(w_qkv[C + r0:C + r0 + 256].T.astype(np.float16)),
            "wvT": np.ascontiguousarray(w_qkv[2 * C + r0:2 * C + r0 + 256].T.astype(np.float16)),
            "wpT": np.ascontiguousarray(w_proj[:, r0:r0 + 256].T.astype(np.float16)),
        })
    return in_maps


def assemble(results):
    attn = np.empty((B, H, N, N), np.float32)
    atv = np.empty((B, N, C), np.float32)
    proj = np.empty((B, N, C), np.float32)
    for c in range(NCORES):
        b, g = divmod(c, 4)
        for j in range(HPC):
            attn[b, HPC * g + j] = results[c]["attnT"][j].T
        atv[b][:, g * 256:(g + 1) * 256] = results[c]["atvT"].T
    for b in range(B):
        acc = results[4 * b]["projT"].astype(np.float32)
        for g in range(1, 4):
            acc += results[4 * b + g]["projT"]
        proj[b] = acc.T
    return attn, atv, proj


def kernel(x, w_qkv, w_proj):
    x = np.asarray(x, dtype=np.float32)
    w_qkv = np.asarray(w_qkv, dtype=np.float32)
    w_proj = np.asarray(w_proj, dtype=np.float32)
    nc = _get_nc()
    in_maps = make_in_maps(x, w_qkv, w_proj)
    res = run_bass_kernel_spmd(nc, in_maps, list(range(NCORES))).results
    return assemble(res)
